# revision 1
# baseline (speedup 1.0000x reference)
"""Trainium2 Bass kernel for sparse_attention scoring + softmax.

Computes, for full inputs:
    enc = encoder_outputs[0]                      # [S=32768, H=1024]
    energies = (enc @ W^T + b) @ hidden           # [S]
    attn = softmax(energies)                      # -> [1, 1, S]

Algebraic restructure: energies = enc @ (W^T @ hidden) + (b . hidden).
The additive constant (b . hidden) is dropped because softmax is invariant
to constant shifts.  The tiny [H] vector v = W^T @ hidden is computed on
host (0.003% of FLOPs); the device streams enc (sequence-parallel across
8 cores), computes per-shard energies with fused DVE multiply-reduce ops,
all-gathers the [S] energies and does the softmax on device.
"""

import sys

sys.path.insert(0, "/opt/trn_rl_repo")

from contextlib import ExitStack

import numpy as np

import concourse.bass as bass
import concourse.bacc as bacc
import concourse.mybir as mybir
import concourse.tile as tile
from concourse.bass_utils import run_bass_kernel_spmd

N_CORES = 8
SEQ = 32768
HID = 1024
SHARD = SEQ // N_CORES  # 4096 seq positions per core

# Main-loop tiling: outer DMA tiles of [128, K*HID] (K seq-row-groups per
# partition slot), processed as K fused multiply-reduce ops of [128, HID]
# each.  The first tiles are small so the DVE starts as soon as possible;
# later tiles are 2 MiB for DMA efficiency.
K_MAX = 4
ENC_BUFS = 6


def tile_schedule(n_col):
    """List of K values (in 128-row units) summing to n_col."""
    ks = []
    ramp = [1, 1, 2]
    for k in ramp:
        if sum(ks) + k <= n_col:
            ks.append(k)
    while sum(ks) < n_col:
        ks.append(min(K_MAX, n_col - sum(ks)))
    return ks


def build_body(nc, tc, enc, vb, ident, ones, out, n_cores=N_CORES, seq=SEQ,
               shard=SHARD):
    f32 = mybir.dt.float32
    mx = mybir.AluOpType.max
    n_col = shard // 128            # energy columns accumulated in SBUF
    seq_f = seq // 128              # free size of the full-seq softmax tile

    ctx = ExitStack()
    cpool = ctx.enter_context(tc.tile_pool(name="cpool", bufs=1))
    iopool = ctx.enter_context(tc.tile_pool(name="iopool", bufs=ENC_BUFS))
    wpool = ctx.enter_context(tc.tile_pool(name="wpool", bufs=2))
    dpool = ctx.enter_context(tc.tile_pool(name="dpool", bufs=1, space="DRAM"))
    pspool = ctx.enter_context(tc.tile_pool(name="pspool", bufs=1, space="PSUM"))

    # --- setup: v (pre-broadcast on host) — emitted FIRST so its DMA and the
    # first enc tile's DMA hit the queues before anything else.
    v_sb = cpool.tile([128, HID], f32)
    nc.sync.dma_start(out=v_sb[:, :], in_=vb[:, :])

    # Early throwaway exp so the ~2.4us ACT_TABLE_LOAD(+drain) runs during
    # the main loop; without it the table load lands on the softmax
    # critical path right before the real exp (seen in every trace).
    warm = wpool.tile([1, 1], f32, tag="warm")
    nc.scalar.activation(
        out=warm[:, :], in_=v_sb[0:1, 0:1],
        func=mybir.ActivationFunctionType.Exp,
        bias=v_sb[0:1, 0:1],
    )

    # Warm-up collective: a tiny AllGather issued up front (hidden under the
    # main loop) so the real one hits a warm ncfw/comm path.  Reads the
    # ident DRAM input directly (no SBUF dependency).
    warm_in = dpool.tile([8], f32)
    warm_out = dpool.tile([8 * n_cores], f32, addr_space="Shared")
    nc.sync.dma_start(out=warm_in.rearrange("(a b) -> a b", a=1),
                      in_=ident[0:1, 0:8])
    nc.gpsimd.collective_compute(
        "AllGather",
        mybir.AluOpType.bypass,
        replica_groups=[list(range(n_cores))],
        ins=[warm_in.opt()],
        outs=[warm_out.opt()],
    )

    # --- main loop: energies[p, j] for shard-local seq = j*128 + p ---
    # The energy transpose + e_loc push happens in two halves: the first
    # half right after column n_col/2 completes (hidden under the loop),
    # only the second half sits on the post-loop critical path.
    e_sb = cpool.tile([128, n_col], f32)
    enc_r = enc.rearrange("(j p) h -> p j h", p=128)   # [128, n_col, HID]

    ident_sb = cpool.tile([128, 128], f32)
    ones_sb = cpool.tile([128, 128], f32)
    e_loc = dpool.tile([shard], f32)
    e_loc_r = e_loc.rearrange("(j p) -> j p", p=128)
    e_all = dpool.tile([seq], f32, addr_space="Shared")
    half = n_col // 2

    def push_energies(lo, hi):
        et_ps = pspool.tile([hi - lo, 128], f32, tag="et", name=f"et_ps_{lo}")
        nc.tensor.transpose(et_ps[:, :], e_sb[:, lo:hi], ident_sb[:, :])
        et_sb = cpool.tile([hi - lo, 128], f32, name=f"et_sb_{lo}")
        nc.vector.tensor_copy(et_sb[:, :], et_ps[:, :])
        nc.sync.dma_start(out=e_loc_r[lo:hi, :], in_=et_sb[:, :])

    j0 = 0
    for t, kt in enumerate(tile_schedule(n_col)):
        buf = iopool.tile([128, K_MAX * HID], f32, tag="enc")
        bufv = buf.rearrange("p (k h) -> p k h", k=K_MAX)
        nc.sync.dma_start(out=bufv[:, 0:kt, :], in_=enc_r[:, j0:j0 + kt, :])
        scratch = wpool.tile([128, HID], f32, tag="scratch")
        for k in range(kt):
            j = j0 + k
            # fused multiply + free-dim-sum: out = (in0 * 1.0) * v,
            # accum_out = sum(out).  (tensor_tensor_reduce crashes trn2 HW
            # under this compile path; scalar_tensor_tensor is equivalent.)
            nc.vector.scalar_tensor_tensor(
                out=scratch[:, :],
                in0=buf[:, k * HID:(k + 1) * HID],
                scalar=1.0,
                in1=v_sb[:, :],
                op0=mybir.AluOpType.mult,
                op1=mybir.AluOpType.mult,
                accum_out=e_sb[:, j:j + 1],
            )
        j0 += kt
        if j0 == half:
            # consts needed by push_energies and the softmax tail; emitted
            # here so their DMAs don't compete with the first enc tiles.
            nc.sync.dma_start(out=ident_sb[:, :], in_=ident[:, :])
            nc.sync.dma_start(out=ones_sb[:, :], in_=ones[:, :])
            push_energies(0, half)

    ones_row = ones_sb[0:1, :]
    ones_col = ones_sb[:, 0:1]
    push_energies(half, n_col)

    nc.gpsimd.collective_compute(
        "AllGather",
        mybir.AluOpType.bypass,
        replica_groups=[list(range(n_cores))],
        ins=[e_loc.opt()],
        outs=[e_all.opt()],
    )

    # --- softmax over the full gathered energies ---
    es = iopool.tile([128, seq_f], f32, tag="es")
    nc.sync.dma_start(out=es[:, :], in_=e_all.rearrange("(p f) -> p f", p=128))

    # per-partition max, then cross-partition max via PE transpose + reduce
    m1 = wpool.tile([128, 1], f32, tag="m1", bufs=1)
    nc.vector.tensor_reduce(
        out=m1[:, :], in_=es[:, :], axis=mybir.AxisListType.X, op=mx,
    )
    m1t_ps = pspool.tile([1, 128], f32, tag="m1t")
    nc.tensor.matmul(m1t_ps[:, :], m1[:, :], ident_sb[:, :],
                     start=True, stop=True)
    gmx = wpool.tile([1, 1], f32, tag="gmx", bufs=1)
    nc.vector.tensor_reduce(
        out=gmx[:, :], in_=m1t_ps[:, :], axis=mybir.AxisListType.X, op=mx,
    )
    # broadcast global max to [128,1] via PE ones-row matmul.
    gm_ps = pspool.tile([128, 1], f32, tag="gm")
    nc.tensor.matmul(gm_ps[:, :], ones_row, gmx[0:1, 0:1],
                     start=True, stop=True)
    ngm = wpool.tile([128, 1], f32, tag="ngm", bufs=1)
    nc.vector.tensor_scalar_mul(ngm[:, :], gm_ps[:, :], -1.0)

    a = iopool.tile([128, seq_f], f32, tag="a")
    ssum = wpool.tile([128, 1], f32, tag="ssum", bufs=1)
    nc.scalar.activation(
        out=a[:, :], in_=es[:, :],
        func=mybir.ActivationFunctionType.Exp,
        bias=ngm[:, :], scale=1.0,
        accum_out=ssum[:, :],
    )
    # global sum: ssum.T @ ones -> [1,1]; reciprocal; broadcast back.
    s_ps = pspool.tile([1, 1], f32, tag="s")
    nc.tensor.matmul(s_ps[:, :], ssum[:, :], ones_col,
                     start=True, stop=True)
    rs = wpool.tile([1, 1], f32, tag="rs", bufs=1)
    nc.vector.reciprocal(rs[:, :], s_ps[:, :])
    r_ps = pspool.tile([128, 1], f32, tag="r")
    nc.tensor.matmul(r_ps[:, :], ones_row, rs[0:1, 0:1],
                     start=True, stop=True)
    r_sb = wpool.tile([128, 1], f32, tag="rsb", bufs=1)
    nc.vector.tensor_copy(r_sb[:, :], r_ps[:, :])

    a2 = iopool.tile([128, seq_f], f32, tag="a2")
    nc.vector.tensor_scalar_mul(a2[:, :], a[:, :], r_sb[:, :])
    nc.sync.dma_start(out=out.rearrange("(p f) -> p f", p=128), in_=a2[:, :])

    ctx.close()


def build_nc(n_cores=N_CORES, seq=SEQ, shard=SHARD, debug=False):
    nc = bacc.Bacc(
        "TRN2",
        target_bir_lowering=False,
        debug=debug,
        num_devices=n_cores,
    )
    enc = nc.dram_tensor("enc", [shard, HID], mybir.dt.float32, kind="ExternalInput")
    vb = nc.dram_tensor("vb", [128, HID], mybir.dt.float32, kind="ExternalInput")
    ident = nc.dram_tensor("ident", [128, 128], mybir.dt.float32, kind="ExternalInput")
    ones = nc.dram_tensor("ones", [128, 128], mybir.dt.float32, kind="ExternalInput")
    out = nc.dram_tensor("attn", [seq], mybir.dt.float32, kind="ExternalOutput")
    with tile.TileContext(nc) as tc:
        build_body(nc, tc, enc.ap(), vb.ap(), ident.ap(), ones.ap(), out.ap(),
                   n_cores=n_cores, seq=seq, shard=shard)
    nc.compile()
    return nc


_NC_CACHE = {}


def _get_nc():
    if "nc" not in _NC_CACHE:
        _NC_CACHE["nc"] = build_nc()
    return _NC_CACHE["nc"]


def make_in_maps(hidden, encoder_outputs, attn_w, attn_b=None, n_cores=N_CORES,
                 shard=SHARD):
    hidden = np.asarray(hidden, dtype=np.float32)
    enc = np.asarray(encoder_outputs, dtype=np.float32)[0]
    w = np.asarray(attn_w, dtype=np.float32)
    v = (w.T @ hidden).astype(np.float32)
    vb = np.ascontiguousarray(np.broadcast_to(v[None, :], (128, v.shape[0])))
    ident = np.eye(128, dtype=np.float32)
    ones = np.ones((128, 128), dtype=np.float32)
    return [
        {
            "enc": np.ascontiguousarray(enc[i * shard:(i + 1) * shard, :]),
            "vb": vb,
            "ident": ident,
            "ones": ones,
        }
        for i in range(n_cores)
    ]


def run(in_maps, trace=False, **kwargs):
    nc = _get_nc()
    return run_bass_kernel_spmd(
        nc, in_maps, core_ids=list(range(N_CORES)), trace=trace, **kwargs
    )


def kernel(**inputs):
    in_maps = make_in_maps(
        inputs["hidden"], inputs["encoder_outputs"], inputs["attn_w"],
        inputs.get("attn_b"),
    )
    res = run(in_maps)
    attn = np.asarray(res.results[0]["attn"], dtype=np.float32).reshape(-1)
    return attn[None, None, :]

